# revision 1
# baseline (speedup 1.0000x reference)
"""Trainium2 Bass kernel for a 4D ConvBlock (conv3^4 -> LN -> GELU -> 1x1 conv -> residual).

Strategy (8 NeuronCores, data-parallel over T with halo 1):
  - Core t computes the full output t-slice out[:, :, t] for BOTH batch samples.
  - Partition layout: 128 SBUF partitions = (sample n)*64 + channel c.
  - conv1 is computed as 81 accumulating PE matmuls (one per 3x3x3x3 kernel
    offset) with BLOCK-DIAGONAL weights [128,128] so both samples ride one
    matmul (K=64 channels would otherwise waste half the 128-wide PE array).
  - Spatial H/W halos come from zero-padded SBUF slices (34x34 per (l) slice,
    padded on host); L halos are handled by skipping out-of-range dl offsets;
    T halos by zero-filled neighbor t-slices on edge cores.
  - Channel-wise LayerNorm stats via tiny matmuls (ones-reduce K=128->M=2 per
    sample), broadcast back with a [2->128] matmul; exact-erf GELU on ACT.
  - conv2 (1x1) is a single block-diagonal matmul; residual read straight from
    the padded input slice.
  - Matmuls run in float32r (TF32, full PE rate). The BIR verifier requires
    every matmul operand's producer to round to f32r, so matmul-feeding tiles
    are DECLARED float32r (DMA'd ones come from f32r DRAM tensors; computed
    ones are written by ACT/DVE ops that round on write). Non-matmul consumers
    read those tiles through a bitcast back to f32.
"""
import os
import sys

os.environ.setdefault("MYCRO_LOCAL_CACHE", "1")
for _p in ("/opt/trn_rl_repo",):
    if os.path.isdir(_p) and _p not in sys.path:
        sys.path.insert(0, _p)

import numpy as np

import concourse.bass as bass
import concourse.tile as tile
from concourse import bacc, mybir
from concourse import bass_utils

# float32 = exact, quarter-rate PE. float32r = TF32, full-rate PE.
MM_DTYPE = os.environ.get("MM_DTYPE", "float32r")
TRACE = os.environ.get("KERNEL_TRACE", "0") == "1"

N, C, T, L, H, W = 2, 64, 8, 8, 32, 32
P = 128
EPS = 1e-5
OFFSETS = [(dt, dl, dh, dw)
           for dt in (-1, 0, 1) for dl in (-1, 0, 1)
           for dh in (-1, 0, 1) for dw in (-1, 0, 1)]

_CACHE = {}
LAST_RESULTS = None


def _build(mm_dtype_str):
    f32 = mybir.dt.float32
    mmdt = getattr(mybir.dt, mm_dtype_str)
    AF = mybir.ActivationFunctionType

    def asf32(ap):
        return ap if ap.dtype == f32 else ap.bitcast(f32)

    nc = bacc.Bacc("TRN2", target_bir_lowering=False, debug=False,
                   enable_asserts=False, num_devices=8)
    xinp = nc.dram_tensor("xinp", [3, P, L, H + 2, W + 2], mmdt,
                          kind="ExternalInput").ap()
    w1c = nc.dram_tensor("w1c", [C, 81, C], mmdt, kind="ExternalInput").ap()
    w2bd = nc.dram_tensor("w2bd", [P, P], mmdt, kind="ExternalInput").ap()
    onesbc = nc.dram_tensor("onesbc", [P, P], mmdt, kind="ExternalInput").ap()
    params = nc.dram_tensor("params", [P, 5], f32, kind="ExternalInput").ap()
    out = nc.dram_tensor("out", [P, L, H, W], f32, kind="ExternalOutput").ap()

    with tile.TileContext(nc) as tc:
        with (
            tc.tile_pool(name="wpool", bufs=1) as wpool,
            tc.tile_pool(name="xpool", bufs=4) as xpool,
            tc.tile_pool(name="work", bufs=2) as work,
            tc.tile_pool(name="ps_acc", bufs=4, space=bass.MemorySpace.PSUM) as ps_acc,
            tc.tile_pool(name="ps_bc", bufs=1, space=bass.MemorySpace.PSUM) as ps_bc,
            tc.tile_pool(name="ps_out", bufs=2, space=bass.MemorySpace.PSUM) as ps_out,
        ):
            w1sb = []

            def emit_chunk(j):
                # Emission order = DMA queue priority: chunk j is emitted
                # right before its first consuming matmul so startup queues
                # drain the truly critical bytes first.
                assert j == len(w1sb)
                w1j = wpool.tile([P, 27, P], mmdt, name=f"w1sb{j}", tag=f"w1sb{j}")
                nc.vector.memset(w1j[0:C, :, C:P].bitcast(f32), 0.0)
                nc.vector.memset(w1j[C:P, :, 0:C].bitcast(f32), 0.0)
                nc.sync.dma_start(w1j[0:C, :, 0:C],
                                  w1c[:, 27 * j: 27 * (j + 1), :])
                nc.sync.dma_start(w1j[C:P, :, C:P],
                                  w1c[:, 27 * j: 27 * (j + 1), :])
                w1sb.append(w1j)

            xs = {}

            def load_one(tb, l):
                xt = xpool.tile([P, H + 2, W + 2], mmdt,
                                name=f"x{tb}_{l}", tag=f"x{tb}")
                # two DMAs per slice -> more queues active during startup
                nc.sync.dma_start(xt[:, 0:17, :], xinp[tb, :, l, 0:17, :])
                nc.sync.dma_start(xt[:, 17:34, :], xinp[tb, :, l, 17:34, :])
                xs[(tb, l)] = xt

            def load_slice(l):
                for tb in range(3):
                    load_one(tb, l)

            def process(l):
                act_os = [o for o, (dt, dl, dh, dw) in enumerate(OFFSETS)
                          if 0 <= l + dl < L]
                act_insts = []
                for half in range(2):
                    h0 = 16 * half
                    acc = ps_acc.tile([P, 16, W], f32,
                                      name=f"acc_{l}_{half}", tag="acc")
                    for i, o in enumerate(act_os):
                        dt, dl, dh, dw = OFFSETS[o]
                        while o // 27 >= len(w1sb):
                            emit_chunk(len(w1sb))
                        rhs = xs[(dt + 1, l + dl)][:, h0 + dh + 1: h0 + dh + 17,
                                                   dw + 1: dw + 33]
                        nc.tensor.matmul(acc[:], w1sb[o // 27][:, o % 27, :], rhs,
                                         start=(i == 0),
                                         stop=(i == len(act_os) - 1))
                    h = work.tile([P, 16, W], mmdt, name=f"h_{l}_{half}", tag="h")
                    nc.vector.tensor_scalar_add(h[:], acc[:], b1_ap)
                    sq = work.tile([P, 16, W], mmdt, name=f"sq_{l}_{half}", tag="sq")
                    nc.vector.tensor_mul(sq[:], asf32(h[:]), asf32(h[:]))
                    bc_mu = ps_bc.tile([P, 16, W], f32,
                                       name=f"bcmu_{l}_{half}", tag="bc_mu")
                    nc.tensor.matmul(bc_mu[:], onsb[:], h[:])
                    bc_e2 = ps_bc.tile([P, 16, W], f32,
                                       name=f"bce2_{l}_{half}", tag="bc_e2")
                    nc.tensor.matmul(bc_e2[:], onsb[:], sq[:])
                    mu_sbf = work.tile([P, 16, W], f32,
                                       name=f"musbf_{l}_{half}", tag="mu_sbf")
                    nc.vector.tensor_copy(mu_sbf[:], bc_mu[:])
                    mu2 = work.tile([P, 16, W], f32,
                                    name=f"mu2_{l}_{half}", tag="mu2")
                    nc.vector.tensor_mul(mu2[:], mu_sbf[:], mu_sbf[:])
                    var = work.tile([P, 16, W], f32,
                                    name=f"var_{l}_{half}", tag="var")
                    nc.vector.tensor_sub(var[:], bc_e2[:], mu2[:])
                    rstd = work.tile([P, 16, W], f32,
                                     name=f"rstd_{l}_{half}", tag="rstd")
                    absr_i = nc.scalar.activation(rstd[:], var[:],
                                                  AF.Abs_reciprocal_sqrt,
                                                  bias=eps_ap, scale=1.0)
                    t1 = work.tile([P, 16, W], f32, name=f"t1_{l}_{half}", tag="t1")
                    nc.vector.tensor_sub(t1[:], asf32(h[:]), mu_sbf[:])
                    t2 = work.tile([P, 16, W], f32, name=f"t2_{l}_{half}", tag="t2")
                    nc.vector.tensor_mul(t2[:], t1[:], rstd[:])
                    g = work.tile([P, 16, W], mmdt, name=f"g_{l}_{half}", tag="g")
                    gelu_i = nc.scalar.activation(g[:], t2[:], AF.Gelu,
                                                  bias=lnb_ap, scale=lnw_ap)
                    act_insts.append((absr_i, gelu_i))
                    ps2 = ps_out.tile([P, 16, W], f32,
                                      name=f"ps2_{l}_{half}", tag="ps2")
                    nc.tensor.matmul(ps2[:], w2sb[:], g[:])
                    o1 = work.tile([P, 16, W], f32, name=f"o1_{l}_{half}", tag="o1")
                    nc.vector.tensor_scalar_add(o1[:], ps2[:], b2_ap)
                    osb = work.tile([P, 16, W], f32,
                                    name=f"osb_{l}_{half}", tag="osb")
                    nc.vector.tensor_add(osb[:], o1[:],
                                         asf32(xs[(1, l)][:, h0 + 1: h0 + 17, 1: 33]))
                    nc.sync.dma_start(out[:, l, h0: h0 + 16, :], osb[:])
                if len(act_insts) == 2:
                    tile.add_dep_helper(
                        act_insts[0][1].ins, act_insts[1][0].ins, sync=True,
                        reason="batch ACT funcs: absr0,absr1,gelu0,gelu1")

            # Emission order == queue-FIFO priority == matmul consumption
            # order: chunk0, then slices tb-major (dt=-1 block reads xp first).
            emit_chunk(0)
            for _tb in range(3):
                load_one(_tb, 0)
                load_one(_tb, 1)
            w2sb = wpool.tile([P, P], mmdt, name="w2sb", tag="w2sb")
            nc.sync.dma_start(w2sb[:], w2bd[:])
            onsb = wpool.tile([P, P], mmdt, name="onsb", tag="onsb")
            nc.sync.dma_start(onsb[:], onesbc[:])
            psb = wpool.tile([P, 5], f32, name="psb", tag="psb")
            nc.sync.dma_start(psb[:], params[:])
            b1_ap = psb[:, 0:1]
            lnw_ap = psb[:, 1:2]
            lnb_ap = psb[:, 2:3]
            b2_ap = psb[:, 3:4]
            eps_ap = psb[:, 4:5]

            process(0)
            for l in range(2, L + 1):
                if l < L:
                    load_slice(l)
                process(l - 1)

    nc.compile()
    return nc


def _get_program():
    key = MM_DTYPE
    if key not in _CACHE:
        _CACHE[key] = _build(key)
    return _CACHE[key]


def _host_prep(x, w1, b1, ln_w, ln_b, w2, b2):
    x = np.ascontiguousarray(np.asarray(x, dtype=np.float32))
    xm = x.reshape(N * C, T, L, H, W)
    # pad H and W by 1 on each side with zeros
    xpad = np.zeros((N * C, T, L, H + 2, W + 2), np.float32)
    xpad[:, :, :, 1:H + 1, 1:W + 1] = xm
    zslice = np.zeros((N * C, L, H + 2, W + 2), np.float32)
    xins = []
    for t in range(T):
        xp = xpad[:, t - 1] if t > 0 else zslice
        xc = xpad[:, t]
        xn = xpad[:, t + 1] if t < T - 1 else zslice
        xins.append(np.ascontiguousarray(np.stack([xp, xc, xn])))

    w1c = np.ascontiguousarray(
        np.asarray(w1, dtype=np.float32).transpose(1, 2, 3, 4, 5, 0)
    ).reshape(C, 81, C)
    w2t = np.asarray(w2, dtype=np.float32).reshape(C, C).T
    w2bd = np.zeros((P, P), np.float32)
    w2bd[:C, :C] = w2t
    w2bd[C:, C:] = w2t
    onesbc = np.zeros((P, P), np.float32)
    onesbc[:C, :C] = 1.0 / C
    onesbc[C:, C:] = 1.0 / C
    params = np.zeros((P, 5), np.float32)
    params[:, 0] = np.tile(np.asarray(b1, dtype=np.float32), 2)
    params[:, 1] = np.tile(np.asarray(ln_w, dtype=np.float32), 2)
    params[:, 2] = np.tile(np.asarray(ln_b, dtype=np.float32), 2)
    params[:, 3] = np.tile(np.asarray(b2, dtype=np.float32), 2)
    params[:, 4] = EPS
    return xins, w1c, w2bd, onesbc, params


def kernel(x, w1, b1, ln_w, ln_b, w2, b2):
    global LAST_RESULTS
    xins, w1c, w2bd, onesbc, params = _host_prep(
        x, w1, b1, ln_w, ln_b, w2, b2)
    nc = _get_program()
    in_maps = [
        {"xinp": xins[t], "w1c": w1c, "w2bd": w2bd, "onesbc": onesbc,
         "params": params}
        for t in range(T)
    ]
    res = bass_utils.run_bass_kernel_spmd(
        nc, in_maps, core_ids=list(range(8)), trace=TRACE)
    LAST_RESULTS = res
    out = np.stack([res.results[t]["out"] for t in range(T)], axis=1)
    return np.ascontiguousarray(out.reshape(N, C, T, L, H, W))



# revision 6
# speedup vs baseline: 1.4114x; 1.4114x over previous
"""Trainium2 Bass kernel for a 4D ConvBlock (conv3^4 -> LN -> GELU -> 1x1 conv -> residual).

Strategy v2 (8 NeuronCores, (sample, t-pair) sharding):
  - Core (n, tp) computes output t-slices t0=2tp, t1=t0+1 for sample n.
  - Partition layout: 128 SBUF partitions = (ts in {0,1})*64 + channel c,
    where ts indexes the two output t-slices.
  - conv1 splits into:
      * INTERIOR taps (inputs t0/t1 -> outputs t0/t1, 4 taps per spatial
        offset): one fully-dense f32r matmul per (dl,dh,dw) - 100% PE
        utilization (vs 50% for the old block-diagonal sample packing).
      * EDGE taps (inputs t0-1 -> out t0, t1+1 -> out t1; 2 taps per
        offset, block-diagonal): packed in PAIRS into fp8e4 DoubleRow
        matmuls (2 fp8 weights/PE cell, K_virtual=256) - 2 edge blocks
        per matmul. Pairing uses strided AP tricks: dl-pairs /dw-pairs
        via step slicing, the one dh-pair per l via a hand-built AP.
        Only ~30% of tap energy runs in fp8 -> rel err ~1.1e-2 (<2e-2).
  - Weights are scaled x256 (fp8 normal range); LayerNorm's affine
    invariance absorbs the scale (eps scaled x256^2). Residual reads the
    exact f32 x tile, so fp8 error only enters through conv1.
  - Software pipeline: stats(l-1) and conv2(l-2) matmuls are emitted
    inside conv(l)'s matmul stream so the PE never waits on the DVE/ACT
    chain. PSUM: acc 2 + stat 4 + conv2 2 = 8 banks exactly.
  - LN chain on DVE reads PSUM directly (no stat copy); bias+residual add
    fused into one GpSimd scalar_tensor_tensor; exact-erf GELU on ACT.
"""
import os
import sys

os.environ.setdefault("MYCRO_LOCAL_CACHE", "1")
for _p in ("/opt/trn_rl_repo",):
    if os.path.isdir(_p) and _p not in sys.path:
        sys.path.insert(0, _p)

import numpy as np
import ml_dtypes

import bass_rust
import concourse.bass as bass
import concourse.tile as tile
from concourse import bacc, mybir
from concourse import bass_utils

MM_DTYPE = "float32r+float8e4(edge)"
TRACE = os.environ.get("KERNEL_TRACE", "0") == "1"

N, C, T, L, H, W = 2, 64, 8, 8, 32, 32
P = 128
EPS = 1e-5
S = 256.0  # fp8 weight scale, folded out by LN affine invariance
FP8 = ml_dtypes.float8_e4m3  # TRN fp8e4 encoding (max normal 240)

_CACHE = {}
LAST_RESULTS = None

# spatial offset index: o = (1+dl)*9 + (1+dh)*3 + (1+dw)
SPATIAL = [(dl, dh, dw) for dl in (-1, 0, 1) for dh in (-1, 0, 1)
           for dw in (-1, 0, 1)]

# DoubleRow pair tables (slot0, slot1 as spatial offsets of EDGE blocks)
PAIRS_INT = (
    [((-1, dh, dw), (1, dh, dw)) for dh in (-1, 0, 1) for dw in (-1, 0, 1)]
    + [((0, dh, -1), (0, dh, 1)) for dh in (-1, 0, 1)]
    + [((0, -1, 0), (0, 1, 0))]
)  # 13 pairs; leftover single: (0,0,0)
PAIRS_LO = [((0, dh, dw), (1, dh, dw)) for dh in (-1, 0, 1)
            for dw in (-1, 0, 1)]  # l=0 (9 pairs)
PAIRS_HI = [((-1, dh, dw), (0, dh, dw)) for dh in (-1, 0, 1)
            for dw in (-1, 0, 1)]  # l=7 (9 pairs)


def _build():
    f32 = mybir.dt.float32
    f32r = mybir.dt.float32r
    f8 = mybir.dt.float8e4
    AF = mybir.ActivationFunctionType
    ALU = mybir.AluOpType
    DR = mybir.MatmulPerfMode.DoubleRow

    def asf32(ap):
        return ap if ap.dtype == f32 else ap.bitcast(f32)

    nc = bacc.Bacc("TRN2", target_bir_lowering=False, debug=False,
                   enable_asserts=False, num_devices=8)
    xf_d = nc.dram_tensor("xf", [P, L, 34, 34], f32r, kind="ExternalInput").ap()
    x8_d = nc.dram_tensor("x8", [P, L, 3, 34, 32], f8, kind="ExternalInput").ap()
    w1f_d = nc.dram_tensor("w1f", [P, 27, P], f32r, kind="ExternalInput").ap()
    wpi_d = nc.dram_tensor("wpi", [P, 13, 2, P], f8, kind="ExternalInput").ap()
    wplo_d = nc.dram_tensor("wplo", [P, 9, 2, P], f8, kind="ExternalInput").ap()
    wphi_d = nc.dram_tensor("wphi", [P, 9, 2, P], f8, kind="ExternalInput").ap()
    wse_d = nc.dram_tensor("wse", [P, P], f8, kind="ExternalInput").ap()
    w2bd_d = nc.dram_tensor("w2bd", [P, P], f32r, kind="ExternalInput").ap()
    ones_d = nc.dram_tensor("onesbc", [P, P], f32r, kind="ExternalInput").ap()
    par_d = nc.dram_tensor("params", [P, 5], f32, kind="ExternalInput").ap()
    out = nc.dram_tensor("out", [P, L, H, W], f32, kind="ExternalOutput").ap()

    with tile.TileContext(nc) as tc:
        with (
            tc.tile_pool(name="wpool", bufs=1) as wpool,
            tc.tile_pool(name="xpool", bufs=1) as xpool,
            tc.tile_pool(name="work", bufs=2) as work,
            tc.tile_pool(name="ps_acc", bufs=1, space=bass.MemorySpace.PSUM) as ps_acc,
            tc.tile_pool(name="ps_bc", bufs=1, space=bass.MemorySpace.PSUM) as ps_bc,
            tc.tile_pool(name="ps_out", bufs=1, space=bass.MemorySpace.PSUM) as ps_out,
        ):
            xft = xpool.tile([P, L, 34, 34], f32r, name="xft", tag="xft")
            xt8 = xpool.tile([P, L, 3, 34, 32], f8, name="xt8", tag="xt8")

            def load(l):
                nc.sync.dma_start(xft[:, l, 0:17, :], xf_d[:, l, 0:17, :])
                nc.sync.dma_start(xft[:, l, 17:34, :], xf_d[:, l, 17:34, :])
                nc.sync.dma_start(xt8[:, l, :, :, :], x8_d[:, l, :, :, :])

            load(0)
            load(1)
            # weights for l=0 processing first
            wplo = wpool.tile([P, 9, 2, P], f8, name="wplo", tag="wplo")
            nc.sync.dma_start(wplo[:], wplo_d[:])
            w1sb = wpool.tile([P, 27, P], f32r, name="w1sb", tag="w1sb")
            nc.sync.dma_start(w1sb[:, 0:9, :], w1f_d[:, 0:9, :])
            nc.sync.dma_start(w1sb[:, 9:18, :], w1f_d[:, 9:18, :])
            nc.sync.dma_start(w1sb[:, 18:27, :], w1f_d[:, 18:27, :])
            load(2)
            wpi = wpool.tile([P, 13, 2, P], f8, name="wpi", tag="wpi")
            nc.sync.dma_start(wpi[:], wpi_d[:])
            wse = wpool.tile([P, P], f8, name="wse", tag="wse")
            nc.sync.dma_start(wse[:], wse_d[:])
            w2sb = wpool.tile([P, P], f32r, name="w2sb", tag="w2sb")
            nc.sync.dma_start(w2sb[:], w2bd_d[:])
            onsb = wpool.tile([P, P], f32r, name="onsb", tag="onsb")
            nc.sync.dma_start(onsb[:], ones_d[:])
            psb = wpool.tile([P, 5], f32, name="psb", tag="psb")
            nc.sync.dma_start(psb[:], par_d[:])
            wphi = wpool.tile([P, 9, 2, P], f8, name="wphi", tag="wphi")
            nc.sync.dma_start(wphi[:], wphi_d[:])

            b1_ap = psb[:, 0:1]
            lnw_ap = psb[:, 1:2]
            lnb_ap = psb[:, 2:3]
            b2_ap = psb[:, 3:4]
            eps_ap = psb[:, 4:5]

            # pipeline state: per (l, half) intermediate tiles
            hts = {}
            accs = {}
            ps2s = {}

            def conv_mms(l):
                """conv1 matmuls for both halves, weight-shared h0/h1."""
                dls = [dl for dl in (-1, 0, 1) if 0 <= l + dl < L]
                units = []  # (kind, payload)
                for dl in dls:
                    for dh in (-1, 0, 1):
                        for dw in (-1, 0, 1):
                            units.append(("I", (dl, dh, dw)))
                if 1 <= l <= L - 2:
                    for i in range(13):
                        units.append(("P", (wpi, i, PAIRS_INT[i])))
                    units.append(("S", None))
                elif l == 0:
                    for i in range(9):
                        units.append(("P", (wplo, i, PAIRS_LO[i])))
                else:
                    for i in range(9):
                        units.append(("P", (wphi, i, PAIRS_HI[i])))

                acc = [ps_acc.tile([P, 16, W], f32, name=f"acc{l}_{h}",
                                   tag=f"acc{h}") for h in (0, 1)]
                accs[l] = acc
                nu = len(units)
                for ui, (kind, pay) in enumerate(units):
                    first, last = ui == 0, ui == nu - 1
                    for h in (0, 1):
                        h0 = 16 * h
                        if kind == "I":
                            dl, dh, dw = pay
                            rhs = xft[:, l + dl, h0 + dh + 1: h0 + dh + 17,
                                      dw + 1: dw + 33]
                            lhsT = w1sb[:, (1 + dl) * 9 + (1 + dh) * 3 + (1 + dw), :]
                            nc.tensor.matmul(acc[h][:], lhsT, rhs,
                                             start=first, stop=last)
                        elif kind == "S":
                            rhs = xt8[:, l, 1, h0 + 1: h0 + 17, :]
                            nc.tensor.matmul(acc[h][:], wse[:], rhs,
                                             start=first, stop=last)
                        else:
                            wt, i, ((dla, dha, dwa), (dlb, dhb, dwb)) = pay
                            if dla != dlb:
                                # dl-pair: step slice on l dim
                                step = dlb - dla
                                rhs = xt8[:, l + dla: l + dlb + 1: step,
                                          dwa + 1, h0 + dha + 1: h0 + dha + 17, :]
                            elif dwa != dwb:
                                # dw-pair: step slice on the dw-copy dim
                                rhs = xt8[:, l, dwa + 1: dwb + 2: (dwb - dwa),
                                          h0 + dha + 1: h0 + dha + 17, :]
                            else:
                                # dh-pair: custom AP, rows h0 and h0+2
                                base = xt8[:, l, dwa + 1,
                                           h0 + dha + 1: h0 + dha + 17, :]
                                rhs = base.copy()
                                pstride = rhs.ap[0][0]
                                rhs.ap = bass_rust.VecI64Pair(
                                    [(pstride, P), ((dhb - dha) * 32, 2),
                                     (32, 16), (1, 32)])
                            nc.tensor.matmul(acc[h][:], wt[:, i, :, :], rhs,
                                             start=first, stop=last,
                                             perf_mode=DR)

            def eltA(l):
                """h = acc + b1', sq = h*h (DVE) for both halves."""
                acc = accs[l]
                for h in (0, 1):
                    ht = work.tile([P, 16, W], f32r, name=f"h{l}_{h}", tag="h")
                    nc.vector.tensor_scalar_add(ht[:], acc[h][:], b1_ap)
                    sq = work.tile([P, 16, W], f32r, name=f"sq{l}_{h}", tag="sq")
                    nc.vector.tensor_mul(sq[:], asf32(ht[:]), asf32(ht[:]))
                    hts[(l, h)] = (ht, sq)

            def stats_mms(l):
                for h in (0, 1):
                    ht, sq = hts[(l, h)]
                    bc_mu = ps_bc.tile([P, 16, W], f32, name=f"bcmu{l}_{h}",
                                       tag=f"bc_mu{h}")
                    nc.tensor.matmul(bc_mu[:], onsb[:], ht[:])
                    bc_e2 = ps_bc.tile([P, 16, W], f32, name=f"bce2{l}_{h}",
                                       tag=f"bc_e2{h}")
                    nc.tensor.matmul(bc_e2[:], onsb[:], sq[:])
                    hts[(l, h)] = (ht, sq, bc_mu, bc_e2)

            def eltB(l):
                """LN normalize + GELU; leaves g tiles for conv2."""
                act_insts = []
                for h in (0, 1):
                    ht, sq, bc_mu, bc_e2 = hts[(l, h)]
                    # PSUM may only appear as the first TT operand, and GpSimd
                    # cannot touch PSUM: stage the mean in SBUF via DVE, then
                    # square it on the Pool engine.
                    mu_sbf = work.tile([P, 16, W], f32, name=f"mus{l}_{h}",
                                       tag="mu_sbf")
                    nc.vector.tensor_copy(mu_sbf[:], bc_mu[:])
                    mu2 = work.tile([P, 16, W], f32, name=f"mu2{l}_{h}", tag="mu2")
                    nc.gpsimd.tensor_mul(mu2[:], mu_sbf[:], mu_sbf[:])
                    var = work.tile([P, 16, W], f32, name=f"var{l}_{h}", tag="var")
                    nc.vector.tensor_sub(var[:], bc_e2[:], mu2[:])
                    rstd = work.tile([P, 16, W], f32, name=f"rstd{l}_{h}",
                                     tag="rstd")
                    absr_i = nc.scalar.activation(rstd[:], var[:],
                                                  AF.Abs_reciprocal_sqrt,
                                                  bias=eps_ap, scale=1.0)
                    t1 = work.tile([P, 16, W], f32, name=f"t1{l}_{h}", tag="t1")
                    nc.vector.tensor_sub(t1[:], asf32(ht[:]), mu_sbf[:])
                    t2 = work.tile([P, 16, W], f32, name=f"t2{l}_{h}", tag="t2")
                    nc.gpsimd.tensor_mul(t2[:], t1[:], rstd[:])
                    g = work.tile([P, 16, W], f32r, name=f"g{l}_{h}", tag="g")
                    gelu_i = nc.scalar.activation(g[:], t2[:], AF.Gelu,
                                                  bias=lnb_ap, scale=lnw_ap)
                    act_insts.append((absr_i, gelu_i))
                    hts[(l, h)] = g
                # batch ACT funcs: absr0,absr1,gelu0,gelu1 (2 table swaps per l)
                tile.add_dep_helper(act_insts[0][1].ins, act_insts[1][0].ins,
                                    sync=True, reason="batch ACT funcs")

            def conv2_mms(l):
                for h in (0, 1):
                    g = hts[(l, h)]
                    ps2 = ps_out.tile([P, 16, W], f32, name=f"ps2{l}_{h}",
                                      tag=f"ps2{h}")
                    nc.tensor.matmul(ps2[:], w2sb[:], g[:])
                    ps2s[(l, h)] = ps2

            def eltC(l):
                """o1 = ps2 + b2 (DVE, PSUM read); osb = o1 + x_res (GpSimd)."""
                for h in (0, 1):
                    h0 = 16 * h
                    ps2 = ps2s.pop((l, h))
                    o1 = work.tile([P, 16, W], f32, name=f"o1{l}_{h}", tag="o1")
                    nc.vector.tensor_scalar_add(o1[:], ps2[:], b2_ap)
                    osb = work.tile([P, 16, W], f32, name=f"osb{l}_{h}",
                                    tag="osb")
                    xres = asf32(xft[:, l, h0 + 1: h0 + 17, 1:33])
                    nc.gpsimd.tensor_add(osb[:], o1[:], xres)
                    nc.sync.dma_start(out[:, l, h0: h0 + 16, :], osb[:])

            # software pipeline: conv(l) | stats(l-1), eltB(l-1) | conv2(l-2)
            for l in range(L):
                conv_mms(l)
                eltA(l)
                if l >= 1:
                    stats_mms(l - 1)
                    eltB(l - 1)
                if l >= 2:
                    conv2_mms(l - 2)
                    eltC(l - 2)
                if l + 3 < L:
                    load(l + 3)
            stats_mms(L - 1)
            eltB(L - 1)
            conv2_mms(L - 2)
            eltC(L - 2)
            conv2_mms(L - 1)
            eltC(L - 1)

    nc.compile()
    return nc


def _get_program():
    if "v2" not in _CACHE:
        _CACHE["v2"] = _build()
    return _CACHE["v2"]


def _edge_block(w8f, dl, dh, dw):
    """[128,128] E-block: diag over ts with kt=0 (ts=0) / kt=2 (ts=1)."""
    eb = np.zeros((P, P), np.float32)
    for ts, kt in ((0, 0), (1, 2)):
        # lhsT[(ts,ci),(ts,co)] = S*w1[co,ci,kt,1+dl,1+dh,1+dw]
        blk = w8f[:, :, kt, 1 + dl, 1 + dh, 1 + dw].T  # [ci, co]
        eb[ts * C:(ts + 1) * C, ts * C:(ts + 1) * C] = blk
    return eb


def _host_prep(x, w1, b1, ln_w, ln_b, w2, b2):
    x = np.ascontiguousarray(np.asarray(x, dtype=np.float32))
    w1 = np.asarray(w1, dtype=np.float32)

    # fp8 quantizations (values on the e4m3 grid, stored as f32 for assembly)
    x8f = x.astype(FP8)
    w8f = (S * w1).astype(FP8).astype(np.float32)

    # padded arrays
    xpad = np.zeros((N, C, T, L, H + 2, W + 2), np.float32)
    xpad[..., 1:H + 1, 1:W + 1] = x
    x8pad = np.zeros((N, C, T, L, H + 2, W + 2), FP8)
    x8pad[..., 1:H + 1, 1:W + 1] = x8f
    zed8 = np.zeros((C, L, H + 2, W + 2), FP8)

    # f32r interior weights [P, 27, P]
    w1f = np.zeros((P, 27, P), np.float32)
    for ts in (0, 1):
        for tso in (0, 1):
            kt = 1 + ts - tso
            # [co, ci, kl,kh,kw] -> [ci, 27, co]
            blk = (S * w1[:, :, kt]).reshape(C, C, 27).transpose(1, 2, 0)
            w1f[ts * C:(ts + 1) * C, :, tso * C:(tso + 1) * C] = blk

    def pairs_tensor(pairs):
        wt = np.zeros((P, len(pairs), 2, P), np.float32)
        for i, (oa, ob) in enumerate(pairs):
            wt[:, i, 0, :] = _edge_block(w8f, *oa)
            wt[:, i, 1, :] = _edge_block(w8f, *ob)
        return wt.astype(FP8)

    wpi = pairs_tensor(PAIRS_INT)
    wplo = pairs_tensor(PAIRS_LO)
    wphi = pairs_tensor(PAIRS_HI)
    wse = _edge_block(w8f, 0, 0, 0).astype(FP8)

    w2t = np.asarray(w2, dtype=np.float32).reshape(C, C).T
    w2bd = np.zeros((P, P), np.float32)
    w2bd[:C, :C] = w2t
    w2bd[C:, C:] = w2t
    onesbc = np.zeros((P, P), np.float32)
    onesbc[:C, :C] = 1.0 / C
    onesbc[C:, C:] = 1.0 / C
    params = np.zeros((P, 5), np.float32)
    params[:, 0] = np.tile(S * np.asarray(b1, dtype=np.float32), 2)
    params[:, 1] = np.tile(np.asarray(ln_w, dtype=np.float32), 2)
    params[:, 2] = np.tile(np.asarray(ln_b, dtype=np.float32), 2)
    params[:, 3] = np.tile(np.asarray(b2, dtype=np.float32), 2)
    params[:, 4] = S * S * EPS

    in_maps = []
    for core in range(8):
        n, tp = core // 4, core % 4
        t0 = 2 * tp
        # exact f32 interior planes [P, L, 34, 34]
        xf_c = np.concatenate([xpad[n, :, t0], xpad[n, :, t0 + 1]], axis=0)
        # fp8 edge planes with 3 w-shifted copies [P, L, 3, 34, 32]
        e0 = x8pad[n, :, t0 - 1] if t0 - 1 >= 0 else zed8
        e1 = x8pad[n, :, t0 + 2] if t0 + 2 < T else zed8
        ecat = np.concatenate([e0, e1], axis=0)  # [P, L, 34, 34]
        x8_c = np.empty((P, L, 3, 34, 32), FP8)
        for k in range(3):
            x8_c[:, :, k, :, :] = ecat[:, :, :, k:k + 32]
        in_maps.append({
            "xf": np.ascontiguousarray(xf_c),
            "x8": np.ascontiguousarray(x8_c),
            "w1f": w1f, "wpi": wpi, "wplo": wplo, "wphi": wphi, "wse": wse,
            "w2bd": w2bd, "onesbc": onesbc, "params": params,
        })
    return in_maps


def kernel(x, w1, b1, ln_w, ln_b, w2, b2):
    global LAST_RESULTS
    in_maps = _host_prep(x, w1, b1, ln_w, ln_b, w2, b2)
    nc = _get_program()
    res = bass_utils.run_bass_kernel_spmd(
        nc, in_maps, core_ids=list(range(8)), trace=TRACE)
    LAST_RESULTS = res
    out = np.empty((N, C, T, L, H, W), np.float32)
    for core in range(8):
        n, tp = core // 4, core % 4
        r = res.results[core]["out"]  # [P, L, H, W]
        out[n, :, 2 * tp] = r[:C]
        out[n, :, 2 * tp + 1] = r[C:]
    return np.ascontiguousarray(out)


# revision 13
# speedup vs baseline: 1.6764x; 1.1877x over previous
"""Trainium2 Bass kernel for a 4D ConvBlock (conv3^4 -> LN -> GELU -> 1x1 conv -> residual).

Strategy v2 (8 NeuronCores, (sample, t-pair) sharding):
  - Core (n, tp) computes output t-slices t0=2tp, t1=t0+1 for sample n.
  - Partition layout: 128 SBUF partitions = (ts in {0,1})*64 + channel c,
    where ts indexes the two output t-slices.
  - conv1 splits into:
      * INTERIOR taps (inputs t0/t1 -> outputs t0/t1, 4 taps per spatial
        offset): one fully-dense f32r matmul per (dl,dh,dw) - 100% PE
        utilization (vs 50% for the old block-diagonal sample packing).
      * EDGE taps (inputs t0-1 -> out t0, t1+1 -> out t1; 2 taps per
        offset, block-diagonal): packed in PAIRS into fp8e4 DoubleRow
        matmuls (2 fp8 weights/PE cell, K_virtual=256) - 2 edge blocks
        per matmul. Pairing uses strided AP tricks: dl-pairs /dw-pairs
        via step slicing, the one dh-pair per l via a hand-built AP.
        Only ~30% of tap energy runs in fp8 -> rel err ~1.1e-2 (<2e-2).
  - Weights are scaled x256 (fp8 normal range); LayerNorm's affine
    invariance absorbs the scale (eps scaled x256^2). Residual reads the
    exact f32 x tile, so fp8 error only enters through conv1.
  - Software pipeline: stats(l-1) and conv2(l-2) matmuls are emitted
    inside conv(l)'s matmul stream so the PE never waits on the DVE/ACT
    chain. PSUM: acc 2 + stat 4 + conv2 2 = 8 banks exactly.
  - LN chain on DVE reads PSUM directly (no stat copy); bias+residual add
    fused into one GpSimd scalar_tensor_tensor; exact-erf GELU on ACT.
"""
import os
import sys

os.environ.setdefault("MYCRO_LOCAL_CACHE", "1")
for _p in ("/opt/trn_rl_repo",):
    if os.path.isdir(_p) and _p not in sys.path:
        sys.path.insert(0, _p)

import numpy as np
import ml_dtypes

import bass_rust
import concourse.bass as bass
import concourse.tile as tile
from concourse import bacc, mybir
from concourse import bass_utils

MM_DTYPE = "float32r+float8e4(edge)"
TRACE = os.environ.get("KERNEL_TRACE", "0") == "1"

N, C, T, L, H, W = 2, 64, 8, 8, 32, 32
P = 128
EPS = 1e-5
S = 256.0  # fp8 weight scale, folded out by LN affine invariance
FP8 = ml_dtypes.float8_e4m3  # TRN fp8e4 encoding (max normal 240)

_CACHE = {}
LAST_RESULTS = None

# spatial offset index: o = (1+dl)*9 + (1+dh)*3 + (1+dw)
SPATIAL = [(dl, dh, dw) for dl in (-1, 0, 1) for dh in (-1, 0, 1)
           for dw in (-1, 0, 1)]

# DoubleRow pair tables (slot0, slot1 as spatial offsets of EDGE blocks)
PAIRS_INT = (
    [((-1, dh, dw), (1, dh, dw)) for dh in (-1, 0, 1) for dw in (-1, 0, 1)]
    + [((0, dh, -1), (0, dh, 1)) for dh in (-1, 0, 1)]
    + [((0, -1, 0), (0, 1, 0))]
)  # 13 pairs; leftover single: (0,0,0)
PAIRS_LO = [((0, dh, dw), (1, dh, dw)) for dh in (-1, 0, 1)
            for dw in (-1, 0, 1)]  # l=0 (9 pairs)
PAIRS_HI = [((-1, dh, dw), (0, dh, dw)) for dh in (-1, 0, 1)
            for dw in (-1, 0, 1)]  # l=7 (9 pairs)


def _build():
    f32 = mybir.dt.float32
    f32r = mybir.dt.float32r
    f8 = mybir.dt.float8e4
    AF = mybir.ActivationFunctionType
    ALU = mybir.AluOpType
    DR = mybir.MatmulPerfMode.DoubleRow

    def asf32(ap):
        return ap if ap.dtype == f32 else ap.bitcast(f32)

    nc = bacc.Bacc("TRN2", target_bir_lowering=False, debug=False,
                   enable_asserts=False, num_devices=8)
    xf_d = nc.dram_tensor("xf", [P, L, 34, 34], f32r, kind="ExternalInput").ap()
    x8_d = nc.dram_tensor("x8", [P, L, 3, 34, 32], f8, kind="ExternalInput").ap()
    w1f_d = nc.dram_tensor("w1f", [P, 27, P], f32r, kind="ExternalInput").ap()
    wpi_d = nc.dram_tensor("wpi", [P, 13, 2, P], f8, kind="ExternalInput").ap()
    wplo_d = nc.dram_tensor("wplo", [P, 9, 2, P], f8, kind="ExternalInput").ap()
    wphi_d = nc.dram_tensor("wphi", [P, 9, 2, P], f8, kind="ExternalInput").ap()
    wse_d = nc.dram_tensor("wse", [P, P], f8, kind="ExternalInput").ap()
    w2bd_d = nc.dram_tensor("w2bd", [P, P], f32r, kind="ExternalInput").ap()
    ones_d = nc.dram_tensor("onesbc", [P, P], f32r, kind="ExternalInput").ap()
    par_d = nc.dram_tensor("params", [P, 5], f32, kind="ExternalInput").ap()
    out = nc.dram_tensor("out", [P, L, H, W], f32, kind="ExternalOutput").ap()

    with tile.TileContext(nc) as tc:
        with (
            tc.tile_pool(name="wpool", bufs=1) as wpool,
            tc.tile_pool(name="xpool", bufs=1) as xpool,
            tc.tile_pool(name="work", bufs=2) as work,
            tc.tile_pool(name="ps_acc", bufs=1, space=bass.MemorySpace.PSUM) as ps_acc,
            tc.tile_pool(name="ps_bc", bufs=1, space=bass.MemorySpace.PSUM) as ps_bc,
            tc.tile_pool(name="ps_out", bufs=1, space=bass.MemorySpace.PSUM) as ps_out,
        ):
            xft = xpool.tile([P, L, 34, 34], f32r, name="xft", tag="xft")
            xt8 = xpool.tile([P, L, 3, 34, 32], f8, name="xt8", tag="xt8")

            def load(l):
                nc.sync.dma_start(xft[:, l, 0:17, :], xf_d[:, l, 0:17, :])
                nc.sync.dma_start(xft[:, l, 17:34, :], xf_d[:, l, 17:34, :])
                nc.sync.dma_start(xt8[:, l, :, :, :], x8_d[:, l, :, :, :])

            load(0)
            load(1)
            # weights for l=0 processing first
            wplo = wpool.tile([P, 9, 2, P], f8, name="wplo", tag="wplo")
            nc.sync.dma_start(wplo[:], wplo_d[:])
            w1sb = wpool.tile([P, 27, P], f32r, name="w1sb", tag="w1sb")
            # l=0 uses dl in {0,+1} = offsets 9..26: load those chunks first
            nc.sync.dma_start(w1sb[:, 9:18, :], w1f_d[:, 9:18, :])
            nc.sync.dma_start(w1sb[:, 18:27, :], w1f_d[:, 18:27, :])
            nc.sync.dma_start(w1sb[:, 0:9, :], w1f_d[:, 0:9, :])
            load(2)
            wpi = wpool.tile([P, 13, 2, P], f8, name="wpi", tag="wpi")
            nc.sync.dma_start(wpi[:], wpi_d[:])
            wse = wpool.tile([P, P], f8, name="wse", tag="wse")
            nc.sync.dma_start(wse[:], wse_d[:])
            w2sb = wpool.tile([P, P], f32r, name="w2sb", tag="w2sb")
            nc.sync.dma_start(w2sb[:], w2bd_d[:])
            onsb = wpool.tile([P, P], f32r, name="onsb", tag="onsb")
            nc.sync.dma_start(onsb[:], ones_d[:])
            psb = wpool.tile([P, 5], f32, name="psb", tag="psb")
            nc.sync.dma_start(psb[:], par_d[:])
            wphi = wpool.tile([P, 9, 2, P], f8, name="wphi", tag="wphi")
            nc.sync.dma_start(wphi[:], wphi_d[:])

            b1_ap = psb[:, 0:1]
            lnw_ap = psb[:, 1:2]
            lnb_ap = psb[:, 2:3]
            b2_ap = psb[:, 3:4]
            eps_ap = psb[:, 4:5]

            # pipeline state: per (l, half) intermediate tiles
            hts = {}
            accs = {}
            ps2s = {}
            # DVE-queue ordering anchors (Tile schedules by readiness, which
            # otherwise runs eltB(l-1) before eltA(l) and delays freeing the
            # single-buffered acc PSUM banks -> 4us PE stall per l)
            last_eltA = {}   # l -> last eltA DVE instruction
            last_eltB = {}   # l -> last eltB DVE instruction

            def conv_mms(l):
                """conv1 matmuls for both halves, weight-shared h0/h1."""
                dls = [dl for dl in (-1, 0, 1) if 0 <= l + dl < L]
                units = []  # (kind, payload)
                for dl in dls:
                    for dh in (-1, 0, 1):
                        for dw in (-1, 0, 1):
                            units.append(("I", (dl, dh, dw)))
                if 1 <= l <= L - 2:
                    for i in range(13):
                        units.append(("P", (wpi, i, PAIRS_INT[i])))
                    units.append(("S", None))
                elif l == 0:
                    for i in range(9):
                        units.append(("P", (wplo, i, PAIRS_LO[i])))
                else:
                    for i in range(9):
                        units.append(("P", (wphi, i, PAIRS_HI[i])))

                acc = [ps_acc.tile([P, 16, W], f32, name=f"acc{l}_{h}",
                                   tag=f"acc{h}") for h in (0, 1)]
                accs[l] = acc
                nu = len(units)
                for ui, (kind, pay) in enumerate(units):
                    first, last = ui == 0, ui == nu - 1
                    for h in (0, 1):
                        h0 = 16 * h
                        if kind == "I":
                            dl, dh, dw = pay
                            rhs = xft[:, l + dl, h0 + dh + 1: h0 + dh + 17,
                                      dw + 1: dw + 33]
                            lhsT = w1sb[:, (1 + dl) * 9 + (1 + dh) * 3 + (1 + dw), :]
                            nc.tensor.matmul(acc[h][:], lhsT, rhs,
                                             start=first, stop=last)
                        elif kind == "S":
                            rhs = xt8[:, l, 1, h0 + 1: h0 + 17, :]
                            nc.tensor.matmul(acc[h][:], wse[:], rhs,
                                             start=first, stop=last)
                        else:
                            wt, i, ((dla, dha, dwa), (dlb, dhb, dwb)) = pay
                            if dla != dlb:
                                # dl-pair: step slice on l dim
                                step = dlb - dla
                                rhs = xt8[:, l + dla: l + dlb + 1: step,
                                          dwa + 1, h0 + dha + 1: h0 + dha + 17, :]
                            elif dwa != dwb:
                                # dw-pair: step slice on the dw-copy dim
                                rhs = xt8[:, l, dwa + 1: dwb + 2: (dwb - dwa),
                                          h0 + dha + 1: h0 + dha + 17, :]
                            else:
                                # dh-pair: custom AP, rows h0 and h0+2
                                base = xt8[:, l, dwa + 1,
                                           h0 + dha + 1: h0 + dha + 17, :]
                                rhs = base.copy()
                                pstride = rhs.ap[0][0]
                                rhs.ap = bass_rust.VecI64Pair(
                                    [(pstride, P), ((dhb - dha) * 32, 2),
                                     (32, 16), (1, 32)])
                            nc.tensor.matmul(acc[h][:], wt[:, i, :, :], rhs,
                                             start=first, stop=last,
                                             perf_mode=DR)

            def eltA(l):
                """h = acc + b1', sq = h*h (DVE) for both halves."""
                acc = accs[l]
                for h in (0, 1):
                    # h survives until t1(l), which is ordered after eltA(l+1):
                    # 4 generations live concurrently
                    ht = work.tile([P, 16, W], f32r, name=f"h{l}_{h}", tag="h",
                                   bufs=4)
                    nc.vector.tensor_scalar_add(ht[:], acc[h][:], b1_ap)
                    sq = work.tile([P, 16, W], f32r, name=f"sq{l}_{h}", tag="sq",
                                   bufs=3)
                    si = nc.vector.tensor_mul(sq[:], asf32(ht[:]), asf32(ht[:]))
                    hts[(l, h)] = (ht, sq)
                last_eltA[l] = si

            def stats_mms(l):
                for h in (0, 1):
                    ht, sq = hts[(l, h)]
                    bc_mu = ps_bc.tile([P, 16, W], f32, name=f"bcmu{l}_{h}",
                                       tag=f"bc_mu{h}")
                    nc.tensor.matmul(bc_mu[:], onsb[:], ht[:])
                    bc_e2 = ps_bc.tile([P, 16, W], f32, name=f"bce2{l}_{h}",
                                       tag=f"bc_e2{h}")
                    nc.tensor.matmul(bc_e2[:], onsb[:], sq[:])
                    hts[(l, h)] = (ht, sq, bc_mu, bc_e2)

            def eltB(l):
                """LN normalize + GELU; leaves g tiles for conv2."""
                act_insts = []
                for h in (0, 1):
                    ht, sq, bc_mu, bc_e2 = hts[(l, h)]
                    # PSUM may only appear as the first TT operand, and GpSimd
                    # cannot touch PSUM: stage the mean in SBUF via DVE, then
                    # square it on the Pool engine.
                    mu_sbf = work.tile([P, 16, W], f32, name=f"mus{l}_{h}",
                                       tag="mu_sbf")
                    ci = nc.vector.tensor_copy(mu_sbf[:], bc_mu[:])
                    if l + 1 in last_eltA:
                        tile.add_dep_helper(ci.ins, last_eltA[l + 1].ins,
                                            sync=True,
                                            reason="DVE order: eltA(l+1) first")
                    mu2 = work.tile([P, 16, W], f32, name=f"mu2{l}_{h}", tag="mu2")
                    nc.gpsimd.tensor_mul(mu2[:], mu_sbf[:], mu_sbf[:])
                    var = work.tile([P, 16, W], f32, name=f"var{l}_{h}", tag="var")
                    nc.vector.tensor_sub(var[:], bc_e2[:], mu2[:])
                    rstd = work.tile([P, 16, W], f32, name=f"rstd{l}_{h}",
                                     tag="rstd")
                    absr_i = nc.scalar.activation(rstd[:], var[:],
                                                  AF.Abs_reciprocal_sqrt,
                                                  bias=eps_ap, scale=1.0)
                    t1 = work.tile([P, 16, W], f32, name=f"t1{l}_{h}", tag="t1")
                    ti = nc.vector.tensor_sub(t1[:], asf32(ht[:]), mu_sbf[:])
                    last_eltB[l] = ti
                    t2 = work.tile([P, 16, W], f32, name=f"t2{l}_{h}", tag="t2")
                    nc.gpsimd.tensor_mul(t2[:], t1[:], rstd[:])
                    g = work.tile([P, 16, W], f32r, name=f"g{l}_{h}", tag="g")
                    gelu_i = nc.scalar.activation(g[:], t2[:], AF.Gelu,
                                                  bias=lnb_ap, scale=lnw_ap)
                    act_insts.append((absr_i, gelu_i))
                    hts[(l, h)] = g
                # batch ACT funcs: absr0,absr1,gelu0,gelu1 (2 table swaps per l)
                tile.add_dep_helper(act_insts[0][1].ins, act_insts[1][0].ins,
                                    sync=True, reason="batch ACT funcs")

            def conv2_mms(l):
                for h in (0, 1):
                    g = hts[(l, h)]
                    ps2 = ps_out.tile([P, 16, W], f32, name=f"ps2{l}_{h}",
                                      tag=f"ps2{h}")
                    nc.tensor.matmul(ps2[:], w2sb[:], g[:])
                    ps2s[(l, h)] = ps2

            def eltC(l):
                """o1 = ps2 + b2 (DVE, PSUM read); osb = o1 + x_res (GpSimd)."""
                for h in (0, 1):
                    h0 = 16 * h
                    ps2 = ps2s.pop((l, h))
                    o1 = work.tile([P, 16, W], f32, name=f"o1{l}_{h}", tag="o1")
                    oi = nc.vector.tensor_scalar_add(o1[:], ps2[:], b2_ap)
                    if l + 1 in last_eltB:
                        tile.add_dep_helper(oi.ins, last_eltB[l + 1].ins,
                                            sync=True,
                                            reason="DVE order: eltB(l+1) first")
                    osb = work.tile([P, 16, W], f32, name=f"osb{l}_{h}",
                                    tag="osb")
                    xres = asf32(xft[:, l, h0 + 1: h0 + 17, 1:33])
                    nc.gpsimd.tensor_add(osb[:], o1[:], xres)
                    nc.sync.dma_start(out[:, l, h0: h0 + 16, :], osb[:])

            # software pipeline: conv(l) | stats(l-1), eltB(l-1) | conv2(l-2)
            for l in range(L):
                conv_mms(l)
                eltA(l)
                if l >= 1:
                    stats_mms(l - 1)
                    eltB(l - 1)
                if l >= 2:
                    conv2_mms(l - 2)
                    eltC(l - 2)
                if l + 3 < L:
                    load(l + 3)
            stats_mms(L - 1)
            eltB(L - 1)
            conv2_mms(L - 2)
            eltC(L - 2)
            conv2_mms(L - 1)
            eltC(L - 1)

    nc.compile()
    return nc


def _get_program():
    if "v2" not in _CACHE:
        _CACHE["v2"] = _build()
    return _CACHE["v2"]


def _edge_block(w8f, dl, dh, dw):
    """[128,128] E-block: diag over ts with kt=0 (ts=0) / kt=2 (ts=1)."""
    eb = np.zeros((P, P), np.float32)
    for ts, kt in ((0, 0), (1, 2)):
        # lhsT[(ts,ci),(ts,co)] = S*w1[co,ci,kt,1+dl,1+dh,1+dw]
        blk = w8f[:, :, kt, 1 + dl, 1 + dh, 1 + dw].T  # [ci, co]
        eb[ts * C:(ts + 1) * C, ts * C:(ts + 1) * C] = blk
    return eb


def _host_prep(x, w1, b1, ln_w, ln_b, w2, b2):
    x = np.ascontiguousarray(np.asarray(x, dtype=np.float32))
    w1 = np.asarray(w1, dtype=np.float32)

    # fp8 quantizations (values on the e4m3 grid, stored as f32 for assembly)
    x8f = x.astype(FP8)
    w8f = (S * w1).astype(FP8).astype(np.float32)

    # padded arrays
    xpad = np.zeros((N, C, T, L, H + 2, W + 2), np.float32)
    xpad[..., 1:H + 1, 1:W + 1] = x
    x8pad = np.zeros((N, C, T, L, H + 2, W + 2), FP8)
    x8pad[..., 1:H + 1, 1:W + 1] = x8f
    zed8 = np.zeros((C, L, H + 2, W + 2), FP8)

    # f32r interior weights [P, 27, P]
    w1f = np.zeros((P, 27, P), np.float32)
    for ts in (0, 1):
        for tso in (0, 1):
            kt = 1 + ts - tso
            # [co, ci, kl,kh,kw] -> [ci, 27, co]
            blk = (S * w1[:, :, kt]).reshape(C, C, 27).transpose(1, 2, 0)
            w1f[ts * C:(ts + 1) * C, :, tso * C:(tso + 1) * C] = blk

    def pairs_tensor(pairs):
        wt = np.zeros((P, len(pairs), 2, P), np.float32)
        for i, (oa, ob) in enumerate(pairs):
            wt[:, i, 0, :] = _edge_block(w8f, *oa)
            wt[:, i, 1, :] = _edge_block(w8f, *ob)
        return wt.astype(FP8)

    wpi = pairs_tensor(PAIRS_INT)
    wplo = pairs_tensor(PAIRS_LO)
    wphi = pairs_tensor(PAIRS_HI)
    wse = _edge_block(w8f, 0, 0, 0).astype(FP8)

    w2t = np.asarray(w2, dtype=np.float32).reshape(C, C).T
    w2bd = np.zeros((P, P), np.float32)
    w2bd[:C, :C] = w2t
    w2bd[C:, C:] = w2t
    onesbc = np.zeros((P, P), np.float32)
    onesbc[:C, :C] = 1.0 / C
    onesbc[C:, C:] = 1.0 / C
    params = np.zeros((P, 5), np.float32)
    params[:, 0] = np.tile(S * np.asarray(b1, dtype=np.float32), 2)
    params[:, 1] = np.tile(np.asarray(ln_w, dtype=np.float32), 2)
    params[:, 2] = np.tile(np.asarray(ln_b, dtype=np.float32), 2)
    params[:, 3] = np.tile(np.asarray(b2, dtype=np.float32), 2)
    params[:, 4] = S * S * EPS

    in_maps = []
    for core in range(8):
        n, tp = core // 4, core % 4
        t0 = 2 * tp
        # exact f32 interior planes [P, L, 34, 34]
        xf_c = np.concatenate([xpad[n, :, t0], xpad[n, :, t0 + 1]], axis=0)
        # fp8 edge planes with 3 w-shifted copies [P, L, 3, 34, 32]
        e0 = x8pad[n, :, t0 - 1] if t0 - 1 >= 0 else zed8
        e1 = x8pad[n, :, t0 + 2] if t0 + 2 < T else zed8
        ecat = np.concatenate([e0, e1], axis=0)  # [P, L, 34, 34]
        x8_c = np.empty((P, L, 3, 34, 32), FP8)
        for k in range(3):
            x8_c[:, :, k, :, :] = ecat[:, :, :, k:k + 32]
        in_maps.append({
            "xf": np.ascontiguousarray(xf_c),
            "x8": np.ascontiguousarray(x8_c),
            "w1f": w1f, "wpi": wpi, "wplo": wplo, "wphi": wphi, "wse": wse,
            "w2bd": w2bd, "onesbc": onesbc, "params": params,
        })
    return in_maps


def kernel(x, w1, b1, ln_w, ln_b, w2, b2):
    global LAST_RESULTS
    in_maps = _host_prep(x, w1, b1, ln_w, ln_b, w2, b2)
    nc = _get_program()
    res = bass_utils.run_bass_kernel_spmd(
        nc, in_maps, core_ids=list(range(8)), trace=TRACE)
    LAST_RESULTS = res
    out = np.empty((N, C, T, L, H, W), np.float32)
    for core in range(8):
        n, tp = core // 4, core % 4
        r = res.results[core]["out"]  # [P, L, H, W]
        out[n, :, 2 * tp] = r[:C]
        out[n, :, 2 * tp + 1] = r[C:]
    return np.ascontiguousarray(out)


# revision 14
# speedup vs baseline: 1.7080x; 1.0188x over previous
"""Trainium2 Bass kernel for a 4D ConvBlock (conv3^4 -> LN -> GELU -> 1x1 conv -> residual).

Strategy v2 (8 NeuronCores, (sample, t-pair) sharding):
  - Core (n, tp) computes output t-slices t0=2tp, t1=t0+1 for sample n.
  - Partition layout: 128 SBUF partitions = (ts in {0,1})*64 + channel c,
    where ts indexes the two output t-slices.
  - conv1 splits into:
      * INTERIOR taps (inputs t0/t1 -> outputs t0/t1, 4 taps per spatial
        offset): one fully-dense f32r matmul per (dl,dh,dw) - 100% PE
        utilization (vs 50% for the old block-diagonal sample packing).
      * EDGE taps (inputs t0-1 -> out t0, t1+1 -> out t1; 2 taps per
        offset, block-diagonal): packed in PAIRS into fp8e4 DoubleRow
        matmuls (2 fp8 weights/PE cell, K_virtual=256) - 2 edge blocks
        per matmul. Pairing uses strided AP tricks: dl-pairs /dw-pairs
        via step slicing, the one dh-pair per l via a hand-built AP.
        Only ~30% of tap energy runs in fp8 -> rel err ~1.1e-2 (<2e-2).
  - Weights are scaled x256 (fp8 normal range); LayerNorm's affine
    invariance absorbs the scale (eps scaled x256^2). Residual reads the
    exact f32 x tile, so fp8 error only enters through conv1.
  - Software pipeline: stats(l-1) and conv2(l-2) matmuls are emitted
    inside conv(l)'s matmul stream so the PE never waits on the DVE/ACT
    chain. PSUM: acc 2 + stat 4 + conv2 2 = 8 banks exactly.
  - LN chain on DVE reads PSUM directly (no stat copy); bias+residual add
    fused into one GpSimd scalar_tensor_tensor; exact-erf GELU on ACT.
"""
import os
import sys

os.environ.setdefault("MYCRO_LOCAL_CACHE", "1")
for _p in ("/opt/trn_rl_repo",):
    if os.path.isdir(_p) and _p not in sys.path:
        sys.path.insert(0, _p)

import numpy as np
import ml_dtypes

import bass_rust
import concourse.bass as bass
import concourse.tile as tile
from concourse import bacc, mybir
from concourse import bass_utils

MM_DTYPE = "float32r+float8e4(edge)"
TRACE = os.environ.get("KERNEL_TRACE", "0") == "1"

N, C, T, L, H, W = 2, 64, 8, 8, 32, 32
P = 128
EPS = 1e-5
S = 256.0  # fp8 weight scale, folded out by LN affine invariance
FP8 = ml_dtypes.float8_e4m3  # TRN fp8e4 encoding (max normal 240)

_CACHE = {}
LAST_RESULTS = None

# spatial offset index: o = (1+dl)*9 + (1+dh)*3 + (1+dw)
SPATIAL = [(dl, dh, dw) for dl in (-1, 0, 1) for dh in (-1, 0, 1)
           for dw in (-1, 0, 1)]

# DoubleRow pair tables (slot0, slot1 as spatial offsets of EDGE blocks)
PAIRS_INT = (
    [((-1, dh, dw), (1, dh, dw)) for dh in (-1, 0, 1) for dw in (-1, 0, 1)]
    + [((0, dh, -1), (0, dh, 1)) for dh in (-1, 0, 1)]
    + [((0, -1, 0), (0, 1, 0))]
)  # 13 pairs; leftover single: (0,0,0)
PAIRS_LO = [((0, dh, dw), (1, dh, dw)) for dh in (-1, 0, 1)
            for dw in (-1, 0, 1)]  # l=0 (9 pairs)
PAIRS_HI = [((-1, dh, dw), (0, dh, dw)) for dh in (-1, 0, 1)
            for dw in (-1, 0, 1)]  # l=7 (9 pairs)


def _build():
    f32 = mybir.dt.float32
    f32r = mybir.dt.float32r
    f8 = mybir.dt.float8e4
    AF = mybir.ActivationFunctionType
    ALU = mybir.AluOpType
    DR = mybir.MatmulPerfMode.DoubleRow

    def asf32(ap):
        return ap if ap.dtype == f32 else ap.bitcast(f32)

    nc = bacc.Bacc("TRN2", target_bir_lowering=False, debug=False,
                   enable_asserts=False, num_devices=8)
    xf_d = nc.dram_tensor("xf", [P, L, 34, 34], f32r, kind="ExternalInput").ap()
    x8_d = nc.dram_tensor("x8", [P, L, 3, 34, 32], f8, kind="ExternalInput").ap()
    w1f_d = nc.dram_tensor("w1f", [P, 27, P], f32r, kind="ExternalInput").ap()
    wpi_d = nc.dram_tensor("wpi", [P, 13, 2, P], f8, kind="ExternalInput").ap()
    wplo_d = nc.dram_tensor("wplo", [P, 9, 2, P], f8, kind="ExternalInput").ap()
    wphi_d = nc.dram_tensor("wphi", [P, 9, 2, P], f8, kind="ExternalInput").ap()
    wse_d = nc.dram_tensor("wse", [P, P], f8, kind="ExternalInput").ap()
    w2bd_d = nc.dram_tensor("w2bd", [P, P], f32r, kind="ExternalInput").ap()
    ones_d = nc.dram_tensor("onesbc", [P, P], f32r, kind="ExternalInput").ap()
    par_d = nc.dram_tensor("params", [P, 5], f32, kind="ExternalInput").ap()
    out = nc.dram_tensor("out", [P, L, H, W], f32, kind="ExternalOutput").ap()

    with tile.TileContext(nc) as tc:
        with (
            tc.tile_pool(name="wpool", bufs=1) as wpool,
            tc.tile_pool(name="xpool", bufs=1) as xpool,
            tc.tile_pool(name="work", bufs=2) as work,
            tc.tile_pool(name="ps_acc", bufs=1, space=bass.MemorySpace.PSUM) as ps_acc,
            tc.tile_pool(name="ps_bc", bufs=1, space=bass.MemorySpace.PSUM) as ps_bc,
            tc.tile_pool(name="ps_out", bufs=1, space=bass.MemorySpace.PSUM) as ps_out,
        ):
            xft = xpool.tile([P, L, 34, 34], f32r, name="xft", tag="xft")
            xt8 = xpool.tile([P, L, 3, 34, 32], f8, name="xt8", tag="xt8")

            def load(l):
                nc.sync.dma_start(xft[:, l, :, :], xf_d[:, l, :, :])
                nc.sync.dma_start(xt8[:, l, :, :, :], x8_d[:, l, :, :, :])

            # startup order = DMA priority: conv(0)'s first units need
            # xf(0) + w1f[9:18] (dl=0 block), then x8(0)+wplo for its DR pairs
            nc.sync.dma_start(xft[:, 0, :, :], xf_d[:, 0, :, :])
            w1sb = wpool.tile([P, 27, P], f32r, name="w1sb", tag="w1sb")
            nc.sync.dma_start(w1sb[:, 9:18, :], w1f_d[:, 9:18, :])
            nc.sync.dma_start(xft[:, 1, :, :], xf_d[:, 1, :, :])
            nc.sync.dma_start(w1sb[:, 18:27, :], w1f_d[:, 18:27, :])
            nc.sync.dma_start(xt8[:, 0, :, :, :], x8_d[:, 0, :, :, :])
            wplo = wpool.tile([P, 9, 2, P], f8, name="wplo", tag="wplo")
            nc.sync.dma_start(wplo[:], wplo_d[:])
            nc.sync.dma_start(xt8[:, 1, :, :, :], x8_d[:, 1, :, :, :])
            nc.sync.dma_start(w1sb[:, 0:9, :], w1f_d[:, 0:9, :])
            load(2)
            wpi = wpool.tile([P, 13, 2, P], f8, name="wpi", tag="wpi")
            nc.sync.dma_start(wpi[:], wpi_d[:])
            wse = wpool.tile([P, P], f8, name="wse", tag="wse")
            nc.sync.dma_start(wse[:], wse_d[:])
            w2sb = wpool.tile([P, P], f32r, name="w2sb", tag="w2sb")
            nc.sync.dma_start(w2sb[:], w2bd_d[:])
            onsb = wpool.tile([P, P], f32r, name="onsb", tag="onsb")
            nc.sync.dma_start(onsb[:], ones_d[:])
            psb = wpool.tile([P, 5], f32, name="psb", tag="psb")
            nc.sync.dma_start(psb[:], par_d[:])
            wphi = wpool.tile([P, 9, 2, P], f8, name="wphi", tag="wphi")
            nc.sync.dma_start(wphi[:], wphi_d[:])

            b1_ap = psb[:, 0:1]
            lnw_ap = psb[:, 1:2]
            lnb_ap = psb[:, 2:3]
            b2_ap = psb[:, 3:4]
            eps_ap = psb[:, 4:5]

            # pipeline state: per (l, half) intermediate tiles
            hts = {}
            accs = {}
            ps2s = {}
            # DVE-queue ordering anchors (Tile schedules by readiness, which
            # otherwise runs eltB(l-1) before eltA(l) and delays freeing the
            # single-buffered acc PSUM banks -> 4us PE stall per l)
            last_eltA = {}   # l -> last eltA DVE instruction
            last_eltB = {}   # l -> last eltB DVE instruction

            def conv_mms(l):
                """conv1 matmuls for both halves, weight-shared h0/h1."""
                dls = [dl for dl in (-1, 0, 1) if 0 <= l + dl < L]
                units = []  # (kind, payload)
                for dl in dls:
                    for dh in (-1, 0, 1):
                        for dw in (-1, 0, 1):
                            units.append(("I", (dl, dh, dw)))
                if 1 <= l <= L - 2:
                    for i in range(13):
                        units.append(("P", (wpi, i, PAIRS_INT[i])))
                    units.append(("S", None))
                elif l == 0:
                    for i in range(9):
                        units.append(("P", (wplo, i, PAIRS_LO[i])))
                else:
                    for i in range(9):
                        units.append(("P", (wphi, i, PAIRS_HI[i])))

                acc = [ps_acc.tile([P, 16, W], f32, name=f"acc{l}_{h}",
                                   tag=f"acc{h}") for h in (0, 1)]
                accs[l] = acc
                nu = len(units)
                for ui, (kind, pay) in enumerate(units):
                    first, last = ui == 0, ui == nu - 1
                    for h in (0, 1):
                        h0 = 16 * h
                        if kind == "I":
                            dl, dh, dw = pay
                            rhs = xft[:, l + dl, h0 + dh + 1: h0 + dh + 17,
                                      dw + 1: dw + 33]
                            lhsT = w1sb[:, (1 + dl) * 9 + (1 + dh) * 3 + (1 + dw), :]
                            nc.tensor.matmul(acc[h][:], lhsT, rhs,
                                             start=first, stop=last)
                        elif kind == "S":
                            rhs = xt8[:, l, 1, h0 + 1: h0 + 17, :]
                            nc.tensor.matmul(acc[h][:], wse[:], rhs,
                                             start=first, stop=last)
                        else:
                            wt, i, ((dla, dha, dwa), (dlb, dhb, dwb)) = pay
                            if dla != dlb:
                                # dl-pair: step slice on l dim
                                step = dlb - dla
                                rhs = xt8[:, l + dla: l + dlb + 1: step,
                                          dwa + 1, h0 + dha + 1: h0 + dha + 17, :]
                            elif dwa != dwb:
                                # dw-pair: step slice on the dw-copy dim
                                rhs = xt8[:, l, dwa + 1: dwb + 2: (dwb - dwa),
                                          h0 + dha + 1: h0 + dha + 17, :]
                            else:
                                # dh-pair: custom AP, rows h0 and h0+2
                                base = xt8[:, l, dwa + 1,
                                           h0 + dha + 1: h0 + dha + 17, :]
                                rhs = base.copy()
                                pstride = rhs.ap[0][0]
                                rhs.ap = bass_rust.VecI64Pair(
                                    [(pstride, P), ((dhb - dha) * 32, 2),
                                     (32, 16), (1, 32)])
                            nc.tensor.matmul(acc[h][:], wt[:, i, :, :], rhs,
                                             start=first, stop=last,
                                             perf_mode=DR)

            def eltA(l):
                """h = acc + b1', sq = h*h (DVE) for both halves."""
                acc = accs[l]
                for h in (0, 1):
                    # h survives until t1(l), which is ordered after eltA(l+1):
                    # 4 generations live concurrently
                    ht = work.tile([P, 16, W], f32r, name=f"h{l}_{h}", tag="h",
                                   bufs=4)
                    nc.vector.tensor_scalar_add(ht[:], acc[h][:], b1_ap)
                    sq = work.tile([P, 16, W], f32r, name=f"sq{l}_{h}", tag="sq",
                                   bufs=3)
                    si = nc.vector.tensor_mul(sq[:], asf32(ht[:]), asf32(ht[:]))
                    hts[(l, h)] = (ht, sq)
                last_eltA[l] = si

            def stats_mms(l):
                for h in (0, 1):
                    ht, sq = hts[(l, h)]
                    bc_mu = ps_bc.tile([P, 16, W], f32, name=f"bcmu{l}_{h}",
                                       tag=f"bc_mu{h}")
                    nc.tensor.matmul(bc_mu[:], onsb[:], ht[:])
                    bc_e2 = ps_bc.tile([P, 16, W], f32, name=f"bce2{l}_{h}",
                                       tag=f"bc_e2{h}")
                    nc.tensor.matmul(bc_e2[:], onsb[:], sq[:])
                    hts[(l, h)] = (ht, sq, bc_mu, bc_e2)

            def eltB(l):
                """LN normalize + GELU; leaves g tiles for conv2."""
                act_insts = []
                for h in (0, 1):
                    ht, sq, bc_mu, bc_e2 = hts[(l, h)]
                    # PSUM may only appear as the first TT operand, and GpSimd
                    # cannot touch PSUM: stage the mean in SBUF via DVE, then
                    # square it on the Pool engine.
                    mu_sbf = work.tile([P, 16, W], f32, name=f"mus{l}_{h}",
                                       tag="mu_sbf")
                    ci = nc.vector.tensor_copy(mu_sbf[:], bc_mu[:])
                    if l + 1 in last_eltA:
                        tile.add_dep_helper(ci.ins, last_eltA[l + 1].ins,
                                            sync=True,
                                            reason="DVE order: eltA(l+1) first")
                    mu2 = work.tile([P, 16, W], f32, name=f"mu2{l}_{h}", tag="mu2")
                    nc.gpsimd.tensor_mul(mu2[:], mu_sbf[:], mu_sbf[:])
                    var = work.tile([P, 16, W], f32, name=f"var{l}_{h}", tag="var")
                    nc.vector.tensor_sub(var[:], bc_e2[:], mu2[:])
                    rstd = work.tile([P, 16, W], f32, name=f"rstd{l}_{h}",
                                     tag="rstd")
                    absr_i = nc.scalar.activation(rstd[:], var[:],
                                                  AF.Abs_reciprocal_sqrt,
                                                  bias=eps_ap, scale=1.0)
                    t1 = work.tile([P, 16, W], f32, name=f"t1{l}_{h}", tag="t1")
                    ti = nc.vector.tensor_sub(t1[:], asf32(ht[:]), mu_sbf[:])
                    last_eltB[l] = ti
                    t2 = work.tile([P, 16, W], f32, name=f"t2{l}_{h}", tag="t2")
                    nc.gpsimd.tensor_mul(t2[:], t1[:], rstd[:])
                    g = work.tile([P, 16, W], f32r, name=f"g{l}_{h}", tag="g")
                    gelu_i = nc.scalar.activation(g[:], t2[:], AF.Gelu,
                                                  bias=lnb_ap, scale=lnw_ap)
                    act_insts.append((absr_i, gelu_i))
                    hts[(l, h)] = g
                # batch ACT funcs: absr0,absr1,gelu0,gelu1 (2 table swaps per l)
                tile.add_dep_helper(act_insts[0][1].ins, act_insts[1][0].ins,
                                    sync=True, reason="batch ACT funcs")

            def conv2_mms(l):
                for h in (0, 1):
                    g = hts[(l, h)]
                    ps2 = ps_out.tile([P, 16, W], f32, name=f"ps2{l}_{h}",
                                      tag=f"ps2{h}")
                    nc.tensor.matmul(ps2[:], w2sb[:], g[:])
                    ps2s[(l, h)] = ps2

            def eltC(l):
                """o1 = ps2 + b2 (DVE, PSUM read); osb = o1 + x_res (GpSimd)."""
                for h in (0, 1):
                    h0 = 16 * h
                    ps2 = ps2s.pop((l, h))
                    o1 = work.tile([P, 16, W], f32, name=f"o1{l}_{h}", tag="o1")
                    oi = nc.vector.tensor_scalar_add(o1[:], ps2[:], b2_ap)
                    if l + 1 in last_eltB:
                        tile.add_dep_helper(oi.ins, last_eltB[l + 1].ins,
                                            sync=True,
                                            reason="DVE order: eltB(l+1) first")
                    osb = work.tile([P, 16, W], f32, name=f"osb{l}_{h}",
                                    tag="osb")
                    xres = asf32(xft[:, l, h0 + 1: h0 + 17, 1:33])
                    nc.gpsimd.tensor_add(osb[:], o1[:], xres)
                    nc.sync.dma_start(out[:, l, h0: h0 + 16, :], osb[:])

            # software pipeline: conv(l) | stats(l-1), eltB(l-1) | conv2(l-2)
            for l in range(L):
                conv_mms(l)
                eltA(l)
                if l >= 1:
                    stats_mms(l - 1)
                    eltB(l - 1)
                if l >= 2:
                    conv2_mms(l - 2)
                    eltC(l - 2)
                if l + 3 < L:
                    load(l + 3)
            stats_mms(L - 1)
            eltB(L - 1)
            conv2_mms(L - 2)
            eltC(L - 2)
            conv2_mms(L - 1)
            eltC(L - 1)

    nc.compile()
    return nc


def _get_program():
    if "v2" not in _CACHE:
        _CACHE["v2"] = _build()
    return _CACHE["v2"]


def _edge_block(w8f, dl, dh, dw):
    """[128,128] E-block: diag over ts with kt=0 (ts=0) / kt=2 (ts=1)."""
    eb = np.zeros((P, P), np.float32)
    for ts, kt in ((0, 0), (1, 2)):
        # lhsT[(ts,ci),(ts,co)] = S*w1[co,ci,kt,1+dl,1+dh,1+dw]
        blk = w8f[:, :, kt, 1 + dl, 1 + dh, 1 + dw].T  # [ci, co]
        eb[ts * C:(ts + 1) * C, ts * C:(ts + 1) * C] = blk
    return eb


def _host_prep(x, w1, b1, ln_w, ln_b, w2, b2):
    x = np.ascontiguousarray(np.asarray(x, dtype=np.float32))
    w1 = np.asarray(w1, dtype=np.float32)

    # fp8 quantizations (values on the e4m3 grid, stored as f32 for assembly)
    x8f = x.astype(FP8)
    w8f = (S * w1).astype(FP8).astype(np.float32)

    # padded arrays
    xpad = np.zeros((N, C, T, L, H + 2, W + 2), np.float32)
    xpad[..., 1:H + 1, 1:W + 1] = x
    x8pad = np.zeros((N, C, T, L, H + 2, W + 2), FP8)
    x8pad[..., 1:H + 1, 1:W + 1] = x8f
    zed8 = np.zeros((C, L, H + 2, W + 2), FP8)

    # f32r interior weights [P, 27, P]
    w1f = np.zeros((P, 27, P), np.float32)
    for ts in (0, 1):
        for tso in (0, 1):
            kt = 1 + ts - tso
            # [co, ci, kl,kh,kw] -> [ci, 27, co]
            blk = (S * w1[:, :, kt]).reshape(C, C, 27).transpose(1, 2, 0)
            w1f[ts * C:(ts + 1) * C, :, tso * C:(tso + 1) * C] = blk

    def pairs_tensor(pairs):
        wt = np.zeros((P, len(pairs), 2, P), np.float32)
        for i, (oa, ob) in enumerate(pairs):
            wt[:, i, 0, :] = _edge_block(w8f, *oa)
            wt[:, i, 1, :] = _edge_block(w8f, *ob)
        return wt.astype(FP8)

    wpi = pairs_tensor(PAIRS_INT)
    wplo = pairs_tensor(PAIRS_LO)
    wphi = pairs_tensor(PAIRS_HI)
    wse = _edge_block(w8f, 0, 0, 0).astype(FP8)

    w2t = np.asarray(w2, dtype=np.float32).reshape(C, C).T
    w2bd = np.zeros((P, P), np.float32)
    w2bd[:C, :C] = w2t
    w2bd[C:, C:] = w2t
    onesbc = np.zeros((P, P), np.float32)
    onesbc[:C, :C] = 1.0 / C
    onesbc[C:, C:] = 1.0 / C
    params = np.zeros((P, 5), np.float32)
    params[:, 0] = np.tile(S * np.asarray(b1, dtype=np.float32), 2)
    params[:, 1] = np.tile(np.asarray(ln_w, dtype=np.float32), 2)
    params[:, 2] = np.tile(np.asarray(ln_b, dtype=np.float32), 2)
    params[:, 3] = np.tile(np.asarray(b2, dtype=np.float32), 2)
    params[:, 4] = S * S * EPS

    in_maps = []
    for core in range(8):
        n, tp = core // 4, core % 4
        t0 = 2 * tp
        # exact f32 interior planes [P, L, 34, 34]
        xf_c = np.concatenate([xpad[n, :, t0], xpad[n, :, t0 + 1]], axis=0)
        # fp8 edge planes with 3 w-shifted copies [P, L, 3, 34, 32]
        e0 = x8pad[n, :, t0 - 1] if t0 - 1 >= 0 else zed8
        e1 = x8pad[n, :, t0 + 2] if t0 + 2 < T else zed8
        ecat = np.concatenate([e0, e1], axis=0)  # [P, L, 34, 34]
        x8_c = np.empty((P, L, 3, 34, 32), FP8)
        for k in range(3):
            x8_c[:, :, k, :, :] = ecat[:, :, :, k:k + 32]
        in_maps.append({
            "xf": np.ascontiguousarray(xf_c),
            "x8": np.ascontiguousarray(x8_c),
            "w1f": w1f, "wpi": wpi, "wplo": wplo, "wphi": wphi, "wse": wse,
            "w2bd": w2bd, "onesbc": onesbc, "params": params,
        })
    return in_maps


def kernel(x, w1, b1, ln_w, ln_b, w2, b2):
    global LAST_RESULTS
    in_maps = _host_prep(x, w1, b1, ln_w, ln_b, w2, b2)
    nc = _get_program()
    res = bass_utils.run_bass_kernel_spmd(
        nc, in_maps, core_ids=list(range(8)), trace=TRACE)
    LAST_RESULTS = res
    out = np.empty((N, C, T, L, H, W), np.float32)
    for core in range(8):
        n, tp = core // 4, core % 4
        r = res.results[core]["out"]  # [P, L, H, W]
        out[n, :, 2 * tp] = r[:C]
        out[n, :, 2 * tp + 1] = r[C:]
    return np.ascontiguousarray(out)
